# revision 15
# baseline (speedup 1.0000x reference)
"""FP8 semi-sparse (2:4) activation linear — Trainium2 Bass/Tile kernel.

Reference semantics:
  Wq, W_scale = rowwise fp8(e4m3fn) quant of weight      [N, K]
  Xq, X_scale = rowwise fp8(e4m3fn) quant of x           [M, K]
  Xsp         = 2:4 sparsify of Xq (keep 2 largest |.| per group of 4,
                ties -> earlier index)
  out         = (Xsp @ Wq^T) * X_scale * W_scale^T  -> bf16

Implementation (v1.2 — overlapped pipeline):
  * Data-parallel over M: each core gets 1024 rows of x + the full weight.
  * Halved-scale fp8 trick (TRN fp8e4 max 240 vs OCP 448): u = RNE(v/2),
    x4 folded into the output scales.
  * X path per 128-row tile: absmax (DVE) -> fp8 quant (ScalarE) -> 2:4
    selection on contiguous magnitude planes (DVE, all >=2x mode) -> byte
    mask AND -> store packed u16 to DRAM.  Per m-HALF: 16 big xbar
    transposes [512,128]u16 -> [128,512] + ScalarE deinterleave into the
    persistent xspT2 [128,16,2,1024] fp8 planes.
  * W path per 128-row tile: absmax (DVE) -> fp8 quant (ScalarE) -> store
    to DRAM (gpsimd SWDGE queue).  Per n-slice: 16 xbar transposes back to
    SBUF; the matmul reads the raw interleaved pairs via a strided rhs AP
    (DoubleRow accepts [K,2,N] with o-stride 1, n-stride 2) — no W
    deinterleave at all.
  * Matmul: fp8 DoubleRow, contraction 256/instr, 16 accumulating into one
    PSUM bank per (slice, m-block).  Epilogue: one DVE scalar_tensor_tensor
    out_bf16 = (psum * sx4[m]) * swb[n].
  * DMA dispatch is spread: bulk loads/stores on the gpsimd (SWDGE) queue,
    transposes split between sync and scalar HWDGE queues, so no single
    sequencer serializes the pipeline.
  * Emission is software-pipelined: W quant interleaves with X selection;
    each (slice, m-block) MM group + epilogue is emitted as soon as its
    inputs exist (2 W slices open during the X phase, the rest stream
    after), keeping PE busy from ~100us in.
"""

import numpy as np

import concourse.bass as bass
import concourse.mybir as mybir
import concourse.tile as tile
from concourse import bacc
from concourse.bass_utils import run_bass_kernel_spmd

P = 128
M_FULL, K_FULL, N_FULL = 8192, 4096, 4096
NCORES = 8
N_SLICE = 512

F32 = mybir.dt.float32
BF16 = mybir.dt.bfloat16
FP8 = mybir.dt.float8e4
U16 = mybir.dt.uint16

AX = mybir.AxisListType.X
OP = mybir.AluOpType
AF = mybir.ActivationFunctionType

SX_CONST = float(np.float32(4.0 / (448.0 * 448.0)))

# W row-tiles quantized alongside each X m-tile (2 slices' worth early).
W_PER_MT = [4, 2, 2, 0, 0, 0, 0, 0]


def build_nc(m_core=M_FULL // NCORES, k=K_FULL, n=N_FULL) -> bass.Bass:
    assert m_core % P == 0 and k % (2 * P) == 0 and n % N_SLICE == 0
    m_tiles = m_core // P          # 8
    kp_tiles = k // (2 * P)        # 16
    n_slices = n // N_SLICE        # 8
    w_tiles = n // P               # 32
    groups = k // 4                # 1024
    ku = k // 2                    # 2048 u16 per row
    mh = m_core // 2               # 512 rows per m-half

    nc = bacc.Bacc()
    x = nc.declare_dram_parameter("x", [m_core, k], F32, isOutput=False)
    w = nc.declare_dram_parameter("weight", [n, k], F32, isOutput=False)
    out = nc.declare_dram_parameter("out", [m_core, n], BF16, isOutput=True)

    with tile.TileContext(nc) as tc:
        with (
            tc.tile_pool(name="dram", bufs=1, space="DRAM") as dpool,
            tc.tile_pool(name="wld", bufs=2) as wldpool,
            tc.tile_pool(name="small", bufs=8) as spool,
            tc.tile_pool(name="wam", bufs=2) as wampool,
            tc.tile_pool(name="q8", bufs=2) as q8pool,
            tc.tile_pool(name="swb", bufs=3) as swbpool,
            tc.tile_pool(name="ob", bufs=3) as obpool,
            tc.tile_pool(name="persist", bufs=1) as perpool,
            tc.tile_pool(name="ps", bufs=1, space="PSUM") as pspool,
        ):
            pools = {}
            xsp_dram = dpool.tile([m_core, k], FP8)
            wq_dram = dpool.tile([n, k], FP8)
            wamax_dram = dpool.tile([n], F32)
            xspT2 = perpool.tile([P, kp_tiles, 2, m_core], FP8)
            sx4 = perpool.tile([P, m_tiles], F32)
            pss = [
                pspool.tile([P, N_SLICE], F32, tag=f"ps{m}", name=f"ps{m}")
                for m in range(m_tiles)
            ]

            def load_x(mt):
                t = pools["xld"].tile([P, k], F32, tag="xin")
                nc.sync.dma_start(t, x[P * mt : P * (mt + 1), :])
                return t

            def load_w(wt_idx):
                t = wldpool.tile([P, k], F32, tag="win")
                nc.gpsimd.dma_start(t, w[P * wt_idx : P * (wt_idx + 1), :])
                return t

            def quant_scale(t, tag, amax_out=None):
                amax = spool.tile([P, 1], F32, tag=f"am{tag}")
                nc.vector.tensor_reduce(
                    amax, t, axis=AX, op=OP.max, apply_absolute_value=True
                )
                if amax_out is None:
                    amax_out = spool.tile([P, 1], F32, tag=f"amc{tag}")
                nc.vector.tensor_scalar_max(amax_out, amax, 1e-12)
                rec = spool.tile([P, 1], F32, tag=f"rec{tag}")
                nc.vector.reciprocal(rec, amax_out)
                g = spool.tile([P, 1], F32, tag=f"g{tag}")
                nc.vector.tensor_scalar_mul(g, rec, 224.0)
                return g

            # ---------------- W tile: quantize + store ----------------
            wam_tiles = {}

            def w_quant(wt_idx):
                ns, j = wt_idx // 4, wt_idx % 4
                if j == 0:
                    wam_tiles[ns] = wampool.tile(
                        [P, 4], F32, tag="wam", name=f"wam{ns}"
                    )
                t = w_loads.pop(wt_idx)
                g = quant_scale(t, "w", amax_out=wam_tiles[ns][:, j : j + 1])
                wq8 = q8pool.tile([P, k], FP8, tag="q8")
                nc.scalar.activation(wq8, t, AF.Copy, scale=g)
                nc.gpsimd.dma_start(wq_dram[P * wt_idx : P * (wt_idx + 1), :], wq8)
                nc.sync.dma_start(
                    wamax_dram[P * wt_idx : P * (wt_idx + 1)],
                    wam_tiles[ns][:, j : j + 1],
                )

            # ---------------- X tile: quantize + 2:4 select ----------------
            def x_tile(mt):
                t = x_loads.pop(mt)
                amax_c = spool.tile([P, 1], F32, tag="amcx")
                g = quant_scale(t, "x", amax_out=amax_c)
                nc.vector.tensor_scalar_mul(sx4[:, mt : mt + 1], amax_c, SX_CONST)
                xq = q8pool.tile([P, k], FP8, tag="q8")
                nc.scalar.activation(xq, t, AF.Copy, scale=g)
                xq16 = xq.bitcast(U16)
                xqv = xq16.rearrange("p (g t) -> p g t", t=2)

                e = pools["sel"].tile([P, 4, groups], U16, tag="e")
                nc.vector.tensor_scalar(
                    e[:, 0, :], xqv[:, :, 0], 0x007F, None, op0=OP.bitwise_and
                )
                nc.vector.tensor_scalar(
                    e[:, 1, :], xqv[:, :, 0], 8, 0x007F,
                    op0=OP.logical_shift_right, op1=OP.bitwise_and,
                )
                nc.vector.tensor_scalar(
                    e[:, 2, :], xqv[:, :, 1], 0x007F, None, op0=OP.bitwise_and
                )
                nc.vector.tensor_scalar(
                    e[:, 3, :], xqv[:, :, 1], 8, 0x007F,
                    op0=OP.logical_shift_right, op1=OP.bitwise_and,
                )

                b6 = pools["sel"].tile([P, 6, groups], U16, tag="b6")
                pairs = [(0, 1), (0, 2), (0, 3), (1, 2), (1, 3), (2, 3)]
                bidx = {}
                for pi, (i, j) in enumerate(pairs):
                    nc.vector.tensor_tensor(
                        b6[:, pi, :], e[:, i, :], e[:, j, :], op=OP.is_ge
                    )
                    bidx[(i, j)] = pi

                def b(i, j):
                    return b6[:, bidx[(i, j)], :]

                kk = pools["sel"].tile([P, 4, groups], BF16, tag="kk")
                s = pools["sel"].tile([P, 2, groups], BF16, tag="s")
                nc.vector.tensor_tensor(s[:, 0, :], b(0, 1), b(0, 2), op=OP.add)
                nc.vector.tensor_tensor(s[:, 0, :], s[:, 0, :], b(0, 3), op=OP.add)
                nc.vector.tensor_scalar(kk[:, 0, :], s[:, 0, :], 2.0, None, op0=OP.is_ge)
                nc.vector.tensor_tensor(s[:, 1, :], b(1, 2), b(1, 3), op=OP.add)
                nc.vector.tensor_tensor(s[:, 1, :], s[:, 1, :], b(0, 1), op=OP.subtract)
                nc.vector.tensor_scalar(kk[:, 1, :], s[:, 1, :], 1.0, None, op0=OP.is_ge)
                nc.vector.tensor_tensor(s[:, 0, :], b(2, 3), b(0, 2), op=OP.subtract)
                nc.vector.tensor_tensor(s[:, 0, :], s[:, 0, :], b(1, 2), op=OP.subtract)
                nc.vector.tensor_scalar(kk[:, 2, :], s[:, 0, :], 0.0, None, op0=OP.is_ge)
                nc.vector.tensor_tensor(s[:, 1, :], b(0, 3), b(1, 3), op=OP.add)
                nc.vector.tensor_tensor(s[:, 1, :], s[:, 1, :], b(2, 3), op=OP.add)
                nc.vector.tensor_scalar(kk[:, 3, :], s[:, 1, :], 1.0, None, op0=OP.is_le)

                # byte mask; e is dead — reuse two planes as u16 scratch
                nc.vector.tensor_scalar(
                    e[:, 0, :], kk[:, 1, :], 65280.0, None, op0=OP.mult
                )
                nc.vector.tensor_scalar(
                    e[:, 1, :], kk[:, 3, :], 65280.0, None, op0=OP.mult
                )
                mask = pools["sel"].tile([P, ku], U16, tag="mask")
                mv = mask.rearrange("p (g t) -> p g t", t=2)
                nc.vector.scalar_tensor_tensor(
                    mv[:, :, 0], kk[:, 0, :], 255.0, e[:, 0, :],
                    op0=OP.mult, op1=OP.add,
                )
                nc.vector.scalar_tensor_tensor(
                    mv[:, :, 1], kk[:, 2, :], 255.0, e[:, 1, :],
                    op0=OP.mult, op1=OP.add,
                )
                xsp = pools["xsp"].tile([P, ku], U16, tag="xsp")
                nc.vector.tensor_tensor(xsp, xq16, mask, op=OP.bitwise_and)
                nc.sync.dma_start(
                    xsp_dram.bitcast(U16)[P * mt : P * (mt + 1), :], xsp
                )

            # ---------------- X half: transpose + deinterleave ----------------
            xsp_u16 = xsp_dram.bitcast(U16)  # [m_core, ku]

            def x_half(h):
                xt = pools["xt"].tile([P, kp_tiles, mh], U16, tag="xt")
                for t_ in range(kp_tiles):
                    nc.sync.dma_start_transpose(
                        xt[:, t_, :],
                        xsp_u16[mh * h : mh * (h + 1), P * t_ : P * (t_ + 1)],
                    )
                xt8 = xt.bitcast(FP8).rearrange("p t (m o) -> p t m o", o=2)
                nc.scalar.activation(
                    xspT2[:, :, 0, mh * h : mh * (h + 1)], xt8[:, :, :, 0], AF.Copy
                )
                nc.scalar.activation(
                    xspT2[:, :, 1, mh * h : mh * (h + 1)], xt8[:, :, :, 1], AF.Copy
                )

            # ---------------- MM groups ----------------
            wq_u16 = wq_dram.bitcast(U16)
            wt_slices = {}
            swb_slices = {}

            def open_slice(ns):
                swb = swbpool.tile([P, N_SLICE], F32, tag="swb")
                nc.sync.dma_start(
                    swb,
                    wamax_dram[N_SLICE * ns : N_SLICE * (ns + 1)]
                    .unsqueeze(0)
                    .to_broadcast([P, N_SLICE]),
                )
                swb_slices[ns] = swb
                wt = pools["wt"].tile([P, kp_tiles, N_SLICE], U16, tag="wt")
                for t_ in range(kp_tiles):
                    nc.sync.dma_start_transpose(
                        wt[:, t_, :],
                        wq_u16[N_SLICE * ns : N_SLICE * (ns + 1), P * t_ : P * (t_ + 1)],
                    )
                wt_slices[ns] = wt

            def mm_group(ns, m):
                wt = wt_slices[ns]
                ps = pss[m]
                for t_ in range(kp_tiles):
                    rhs = (
                        wt[:, t_, :]
                        .bitcast(FP8)
                        .rearrange("p (n o) -> p o n", o=2)
                    )
                    nc.tensor.matmul(
                        ps,
                        lhsT=xspT2[:, t_, :, P * m : P * (m + 1)],
                        rhs=rhs,
                        perf_mode=mybir.MatmulPerfMode.DoubleRow,
                        start=(t_ == 0),
                        stop=(t_ == kp_tiles - 1),
                    )
                ob = obpool.tile([P, N_SLICE], BF16, tag="ob")
                nc.vector.scalar_tensor_tensor(
                    ob, ps, sx4[:, m : m + 1], swb_slices[ns],
                    op0=OP.mult, op1=OP.mult,
                )
                nc.gpsimd.dma_start(
                    out[P * m : P * (m + 1), N_SLICE * ns : N_SLICE * (ns + 1)], ob
                )

            # ---------------- fused emission schedule ----------------
            x_loads, w_loads = {}, {}

            emitted = set()
            opened = set()

            def emit_ready(rx, rw):
                for ns in range(rw + 1):
                    if ns not in opened:
                        open_slice(ns)
                        opened.add(ns)
                    for m in range(rx + 1):
                        if (ns, m) not in emitted:
                            mm_group(ns, m)
                            emitted.add((ns, m))

            wseq = 0
            rx = -1
            # X phase: its pools (selection tmps, x loads, staging) live only
            # here; 2 W slices are quantized + matmul'd alongside.
            with (
                tc.tile_pool(name="xld", bufs=2) as _xld,
                tc.tile_pool(name="sel", bufs=1) as _sel,
                tc.tile_pool(name="xsp", bufs=2) as _xsp,
                tc.tile_pool(name="xt", bufs=1) as _xt,
                tc.tile_pool(name="wtA", bufs=2) as _wtA,
            ):
                pools.update(xld=_xld, sel=_sel, xsp=_xsp, xt=_xt, wt=_wtA)
                for j in range(2):
                    w_loads[j] = load_w(j)
                x_loads[0] = load_x(0)
                for mt in range(m_tiles):
                    if mt + 1 < m_tiles:
                        x_loads[mt + 1] = load_x(mt + 1)
                    x_tile(mt)
                    if mt == 3:
                        x_half(0)
                        rx = 3
                    elif mt == 7:
                        x_half(1)
                        rx = 7
                    for _ in range(W_PER_MT[mt]):
                        w_quant(wseq)
                        if wseq + 2 < w_tiles:
                            w_loads[wseq + 2] = load_w(wseq + 2)
                        wseq += 1
                    emit_ready(rx, wseq // 4 - 1)

            # post-X: with the X pools freed, the W-transpose pool gets
            # depth 3, and MM/epilogue emission lags quantization by one
            # slice so the in-order DVE stream never stalls on a slice's
            # quant->store->transpose->MM chain.
            with tc.tile_pool(name="wtB", bufs=3) as _wtB:
                pools["wt"] = _wtB
                while wseq < w_tiles:
                    for _ in range(4):
                        w_quant(wseq)
                        if wseq + 2 < w_tiles:
                            w_loads[wseq + 2] = load_w(wseq + 2)
                        wseq += 1
                    ns_q = wseq // 4 - 1      # slice just quantized
                    if ns_q - 1 >= 0 and ns_q - 1 not in opened:
                        open_slice(ns_q - 1)  # transposes one slice behind
                        opened.add(ns_q - 1)
                    emit_ready(rx, ns_q - 2)  # MMs+epilogues two behind
                for ns_left in range(n_slices):
                    if ns_left not in opened:
                        open_slice(ns_left)
                        opened.add(ns_left)
                    emit_ready(rx, ns_left)

    return nc


_NC = None


def kernel(x: np.ndarray, weight: np.ndarray) -> np.ndarray:
    global _NC
    if _NC is None:
        _NC = build_nc()
        _NC.finalize()
    x = np.ascontiguousarray(x, dtype=np.float32)
    weight = np.ascontiguousarray(weight, dtype=np.float32)
    m_core = M_FULL // NCORES
    in_maps = [
        {"x": x[c * m_core : (c + 1) * m_core], "weight": weight}
        for c in range(NCORES)
    ]
    res = run_bass_kernel_spmd(_NC, in_maps, list(range(NCORES)))
    return np.concatenate([res.results[c]["out"] for c in range(NCORES)], axis=0)


# revision 16
# speedup vs baseline: 1.5082x; 1.5082x over previous
"""FP8 semi-sparse (2:4) activation linear — Trainium2 Bass/Tile kernel.

Reference semantics:
  Wq, W_scale = rowwise fp8(e4m3fn) quant of weight      [N, K]
  Xq, X_scale = rowwise fp8(e4m3fn) quant of x           [M, K]
  Xsp         = 2:4 sparsify of Xq (keep 2 largest |.| per group of 4,
                ties -> earlier index)
  out         = (Xsp @ Wq^T) * X_scale * W_scale^T  -> bf16

Implementation (v2 — collective weight quantization):
  * Data-parallel over M (1024 x-rows/core) for the activation path; the
    WEIGHT path is sharded over cores: core c quantizes + transposes only W
    rows [512c, 512c+512), then an HBM AllGather shares the packed-fp8
    *transposed* weight (16 MB total) and row absmaxes with everyone.
    This removes 7/8 of the weight DVE/ScalarE/DMA work per core and all
    matmul-phase transposes (replaced by one 2 MB load per n-slice).
  * Halved-scale fp8 (TRN fp8e4 max 240 vs OCP 448): u = RNE(v/2), the x4
    folds into the output scales.
  * X path per 128-row tile: absmax (DVE) -> fp8 quant (ScalarE) -> 2:4
    selection on contiguous magnitude planes (DVE, >=2x modes) -> byte mask
    AND -> packed u16 to DRAM.  Per m-half: 16 xbar transposes
    [512,128]u16 -> [128,512] + ScalarE deinterleave into the persistent
    xspT2 [128,16,2,1024] fp8 k-parity planes.
  * Matmul: fp8 DoubleRow, contraction 256/instr; the rhs reads the raw
    interleaved pair layout via a strided [K,2,N] view (o-stride 1,
    n-stride 2) — no weight deinterleave.  16 k-pair matmuls accumulate
    into one PSUM bank per (n-slice, m-block) group.
  * Epilogue: one DVE scalar_tensor_tensor: out_bf16 = (psum*sx4[m])*swb.
  * Emission is software-pipelined: the own-slice W phase runs first (its
    gather overlaps the X phase), m-blocks 0-3 of every n-slice are matmul'd
    while X tiles 4-7 are still being selected, and the m 4-7 sweep streams
    after with the X-phase pools freed.
"""

import dataclasses

import numpy as np

import concourse.bass as bass
import concourse.mybir as mybir
import concourse.tile as tile
from concourse import bacc
from concourse.bass_utils import run_bass_kernel_spmd

P = 128
M_FULL, K_FULL, N_FULL = 8192, 4096, 4096
NCORES = 8
N_SLICE = 512

F32 = mybir.dt.float32
BF16 = mybir.dt.bfloat16
FP8 = mybir.dt.float8e4
U16 = mybir.dt.uint16

AX = mybir.AxisListType.X
OP = mybir.AluOpType
AF = mybir.ActivationFunctionType

SX_CONST = float(np.float32(4.0 / (448.0 * 448.0)))


def build_nc(m_core=M_FULL // NCORES, k=K_FULL, n=N_FULL) -> bass.Bass:
    assert m_core % P == 0 and k % (2 * P) == 0 and n % N_SLICE == 0
    m_tiles = m_core // P          # 8
    kp_tiles = k // (2 * P)        # 16
    n_slices = n // N_SLICE        # 8
    groups = k // 4                # 1024
    ku = k // 2                    # 2048 u16 per row
    mh = m_core // 2               # 512 rows per m-half
    kw = kp_tiles * N_SLICE        # 8192 u16 per partition of one wt slice

    nc = bacc.Bacc(num_devices=NCORES)
    x = nc.declare_dram_parameter("x", [m_core, k], F32, isOutput=False)
    w = nc.declare_dram_parameter("weight", [n, k], F32, isOutput=False)
    out = nc.declare_dram_parameter("out", [m_core, n], BF16, isOutput=True)

    # collective buffers (HBM). inputs Local, outputs Shared.
    wq_own = nc.dram_tensor("wq_own", [N_SLICE, k], FP8)
    wtT_own = nc.dram_tensor("wtT_own", [P, kw], U16)
    wamax_own = nc.dram_tensor("wamax_own", [N_SLICE], F32)
    wtT_all = nc.dram_tensor("wtT_all", [n_slices, P, kw], U16, addr_space="Shared")
    wamax_all = nc.dram_tensor(
        "wamax_all", [n_slices, N_SLICE], F32, addr_space="Shared"
    )

    with tile.TileContext(nc) as tc:
        with (
            tc.tile_pool(name="dram", bufs=1, space="DRAM") as dpool,
            tc.tile_pool(name="small", bufs=8) as spool,
            tc.tile_pool(name="xld", bufs=2) as xldpool,
            tc.tile_pool(name="q8", bufs=2) as q8pool,
            tc.tile_pool(name="ob", bufs=3) as obpool,
            tc.tile_pool(name="persist", bufs=1) as perpool,
            tc.tile_pool(name="ps", bufs=1, space="PSUM") as pspool,
        ):
            xsp_dram = dpool.tile([m_core, k], FP8)
            xspT2 = perpool.tile([P, kp_tiles, 2, m_core], FP8)
            sx4 = perpool.tile([P, m_tiles], F32)
            swb_all = perpool.tile([P, n_slices, N_SLICE], F32)
            pss = [
                pspool.tile([P, N_SLICE], F32, tag=f"ps{m}", name=f"ps{m}")
                for m in range(m_tiles)
            ]
            pools = {}

            def quant_scale(t, tag, amax_out=None):
                amax = spool.tile([P, 1], F32, tag=f"am{tag}")
                nc.vector.tensor_reduce(
                    amax, t, axis=AX, op=OP.max, apply_absolute_value=True
                )
                if amax_out is None:
                    amax_out = spool.tile([P, 1], F32, tag=f"amc{tag}")
                nc.vector.tensor_scalar_max(amax_out, amax, 1e-12)
                rec = spool.tile([P, 1], F32, tag=f"rec{tag}")
                nc.vector.reciprocal(rec, amax_out)
                g = spool.tile([P, 1], F32, tag=f"g{tag}")
                nc.vector.tensor_scalar_mul(g, rec, 224.0)
                return g

            # -------- own-slice W phase: quant 512 rows, transpose, gather ----
            def w_own():
                pid_off = nc.partition_id() * (N_SLICE * k)
                for j in range(4):
                    t = pools["wld"].tile([P, k], F32, tag="win")
                    src = dataclasses.replace(
                        w[0:P, :], offset=pid_off + j * (P * k)
                    )
                    nc.sync.dma_start(t, src)
                    amc = spool.tile([P, 1], F32, tag=f"amcw{j}")
                    g = quant_scale(t, "w", amax_out=amc)
                    wq8 = q8pool.tile([P, k], FP8, tag="q8")
                    nc.scalar.activation(wq8, t, AF.Copy, scale=g)
                    nc.gpsimd.dma_start(wq_own[P * j : P * (j + 1), :], wq8)
                    nc.sync.dma_start(wamax_own[P * j : P * (j + 1)], amc)
                # transpose own 512 rows -> [128, 16, 512] u16, store packed
                wq_own_u16 = wq_own[:, :].bitcast(U16)  # [512, ku]
                wtT = pools["wtT"].tile([P, kp_tiles, N_SLICE], U16, tag="wtT")
                for t_ in range(kp_tiles):
                    nc.sync.dma_start_transpose(
                        wtT[:, t_, :], wq_own_u16[:, P * t_ : P * (t_ + 1)]
                    )
                nc.sync.dma_start(
                    wtT_own[:, :], wtT.rearrange("p t n -> p (t n)")
                )
                nc.gpsimd.collective_compute(
                    "AllGather",
                    mybir.AluOpType.bypass,
                    replica_groups=[list(range(NCORES))],
                    ins=[wtT_own[:, :].opt()],
                    outs=[wtT_all[:, :, :].opt()],
                )
                nc.gpsimd.collective_compute(
                    "AllGather",
                    mybir.AluOpType.bypass,
                    replica_groups=[list(range(NCORES))],
                    ins=[wamax_own[:].opt()],
                    outs=[wamax_all[:, :].opt()],
                )
                # broadcast all slice amaxes into SBUF once
                for ns in range(n_slices):
                    nc.sync.dma_start(
                        swb_all[:, ns, :],
                        wamax_all[ns, :].unsqueeze(0).to_broadcast([P, N_SLICE]),
                    )

            # ---------------- X tile: quantize + 2:4 select ----------------
            def load_x(mt):
                t = xldpool.tile([P, k], F32, tag="xin")
                nc.sync.dma_start(t, x[P * mt : P * (mt + 1), :])
                return t

            def x_tile(mt):
                t = x_loads.pop(mt)
                amax_c = spool.tile([P, 1], F32, tag="amcx")
                g = quant_scale(t, "x", amax_out=amax_c)
                nc.vector.tensor_scalar_mul(sx4[:, mt : mt + 1], amax_c, SX_CONST)
                xq = q8pool.tile([P, k], FP8, tag="q8")
                nc.scalar.activation(xq, t, AF.Copy, scale=g)
                xq16 = xq.bitcast(U16)
                xqv = xq16.rearrange("p (g t) -> p g t", t=2)

                e = pools["sel"].tile([P, 4, groups], U16, tag="e")
                nc.vector.tensor_scalar(
                    e[:, 0, :], xqv[:, :, 0], 0x007F, None, op0=OP.bitwise_and
                )
                nc.vector.tensor_scalar(
                    e[:, 1, :], xqv[:, :, 0], 8, 0x007F,
                    op0=OP.logical_shift_right, op1=OP.bitwise_and,
                )
                nc.vector.tensor_scalar(
                    e[:, 2, :], xqv[:, :, 1], 0x007F, None, op0=OP.bitwise_and
                )
                nc.vector.tensor_scalar(
                    e[:, 3, :], xqv[:, :, 1], 8, 0x007F,
                    op0=OP.logical_shift_right, op1=OP.bitwise_and,
                )

                b6 = pools["sel"].tile([P, 6, groups], U16, tag="b6")
                pairs = [(0, 1), (0, 2), (0, 3), (1, 2), (1, 3), (2, 3)]
                bidx = {}
                for pi, (i, j) in enumerate(pairs):
                    nc.vector.tensor_tensor(
                        b6[:, pi, :], e[:, i, :], e[:, j, :], op=OP.is_ge
                    )
                    bidx[(i, j)] = pi

                def b(i, j):
                    return b6[:, bidx[(i, j)], :]

                kk = pools["sel"].tile([P, 4, groups], BF16, tag="kk")
                s = pools["sel"].tile([P, 2, groups], BF16, tag="s")
                nc.vector.tensor_tensor(s[:, 0, :], b(0, 1), b(0, 2), op=OP.add)
                nc.vector.tensor_tensor(s[:, 0, :], s[:, 0, :], b(0, 3), op=OP.add)
                nc.vector.tensor_scalar(kk[:, 0, :], s[:, 0, :], 2.0, None, op0=OP.is_ge)
                nc.vector.tensor_tensor(s[:, 1, :], b(1, 2), b(1, 3), op=OP.add)
                nc.vector.tensor_tensor(s[:, 1, :], s[:, 1, :], b(0, 1), op=OP.subtract)
                nc.vector.tensor_scalar(kk[:, 1, :], s[:, 1, :], 1.0, None, op0=OP.is_ge)
                nc.vector.tensor_tensor(s[:, 0, :], b(2, 3), b(0, 2), op=OP.subtract)
                nc.vector.tensor_tensor(s[:, 0, :], s[:, 0, :], b(1, 2), op=OP.subtract)
                nc.vector.tensor_scalar(kk[:, 2, :], s[:, 0, :], 0.0, None, op0=OP.is_ge)
                nc.vector.tensor_tensor(s[:, 1, :], b(0, 3), b(1, 3), op=OP.add)
                nc.vector.tensor_tensor(s[:, 1, :], s[:, 1, :], b(2, 3), op=OP.add)
                nc.vector.tensor_scalar(kk[:, 3, :], s[:, 1, :], 1.0, None, op0=OP.is_le)

                nc.vector.tensor_scalar(
                    e[:, 0, :], kk[:, 1, :], 65280.0, None, op0=OP.mult
                )
                nc.vector.tensor_scalar(
                    e[:, 1, :], kk[:, 3, :], 65280.0, None, op0=OP.mult
                )
                mask = pools["sel"].tile([P, ku], U16, tag="mask")
                mv = mask.rearrange("p (g t) -> p g t", t=2)
                nc.vector.scalar_tensor_tensor(
                    mv[:, :, 0], kk[:, 0, :], 255.0, e[:, 0, :],
                    op0=OP.mult, op1=OP.add,
                )
                nc.vector.scalar_tensor_tensor(
                    mv[:, :, 1], kk[:, 2, :], 255.0, e[:, 1, :],
                    op0=OP.mult, op1=OP.add,
                )
                xsp = pools["xsp"].tile([P, ku], U16, tag="xsp")
                nc.vector.tensor_tensor(xsp, xq16, mask, op=OP.bitwise_and)
                nc.sync.dma_start(
                    xsp_dram.bitcast(U16)[P * mt : P * (mt + 1), :], xsp
                )

            # ---------------- X half: transpose + deinterleave ----------------
            xsp_u16 = xsp_dram.bitcast(U16)

            def x_half(h):
                xt = pools["xt"].tile([P, kp_tiles, mh], U16, tag="xt")
                for t_ in range(kp_tiles):
                    nc.sync.dma_start_transpose(
                        xt[:, t_, :],
                        xsp_u16[mh * h : mh * (h + 1), P * t_ : P * (t_ + 1)],
                    )
                xt8 = xt.bitcast(FP8).rearrange("p t (m o) -> p t m o", o=2)
                nc.scalar.activation(
                    xspT2[:, :, 0, mh * h : mh * (h + 1)], xt8[:, :, :, 0], AF.Copy
                )
                nc.scalar.activation(
                    xspT2[:, :, 1, mh * h : mh * (h + 1)], xt8[:, :, :, 1], AF.Copy
                )

            # ---------------- MM groups ----------------
            wt_slices = {}

            def load_wt(ns):
                wt = pools["wt"].tile([P, kp_tiles, N_SLICE], U16, tag="wt")
                nc.gpsimd.dma_start(
                    wt.rearrange("p t n -> p (t n)"), wtT_all[ns, :, :]
                )
                wt_slices[ns] = wt

            def mm_group(ns, m):
                wt = wt_slices[ns]
                ps = pss[m]
                for t_ in range(kp_tiles):
                    rhs = (
                        wt[:, t_, :]
                        .bitcast(FP8)
                        .rearrange("p (n o) -> p o n", o=2)
                    )
                    nc.tensor.matmul(
                        ps,
                        lhsT=xspT2[:, t_, :, P * m : P * (m + 1)],
                        rhs=rhs,
                        perf_mode=mybir.MatmulPerfMode.DoubleRow,
                        start=(t_ == 0),
                        stop=(t_ == kp_tiles - 1),
                    )
                ob = obpool.tile([P, N_SLICE], BF16, tag="ob")
                nc.vector.scalar_tensor_tensor(
                    ob, ps, sx4[:, m : m + 1], swb_all[:, ns, :],
                    op0=OP.mult, op1=OP.mult,
                )
                nc.gpsimd.dma_start(
                    out[P * m : P * (m + 1), N_SLICE * ns : N_SLICE * (ns + 1)], ob
                )

            # ---------------- emission ----------------
            x_loads = {}
            x_loads[0] = load_x(0)

            # prologue: own W slice (gather overlaps the X phase)
            with (
                tc.tile_pool(name="wld", bufs=2) as _wld,
                tc.tile_pool(name="wtT", bufs=1) as _wtT,
            ):
                pools.update(wld=_wld, wtT=_wtT)
                w_own()

            # X phase + the m 0-3 sweep (2 n-slices per X tile from mt3 on)
            with (
                tc.tile_pool(name="sel", bufs=1) as _sel,
                tc.tile_pool(name="xsp", bufs=2) as _xsp,
                tc.tile_pool(name="xt", bufs=1) as _xt,
                tc.tile_pool(name="wtA", bufs=2) as _wtA,
            ):
                pools.update(sel=_sel, xsp=_xsp, xt=_xt, wt=_wtA)
                for mt in range(m_tiles):
                    if mt + 1 < m_tiles:
                        x_loads[mt + 1] = load_x(mt + 1)
                    x_tile(mt)
                    if mt == 3:
                        x_half(0)
                    elif mt == 7:
                        x_half(1)
                    if mt >= 3:
                        for ns in (2 * (mt - 3), 2 * (mt - 3) + 1):
                            if ns < n_slices:
                                load_wt(ns)
                                for m in range(4):
                                    mm_group(ns, m)

            # post-X: m 4-7 sweep over all slices (wt reloaded, deeper pool)
            with tc.tile_pool(name="wtB", bufs=3) as _wtB:
                pools["wt"] = _wtB
                for ns in range(n_slices):
                    load_wt(ns)
                    for m in range(4, m_tiles):
                        mm_group(ns, m)

    return nc


_NC = None


def kernel(x: np.ndarray, weight: np.ndarray) -> np.ndarray:
    global _NC
    if _NC is None:
        _NC = build_nc()
        _NC.finalize()
    x = np.ascontiguousarray(x, dtype=np.float32)
    weight = np.ascontiguousarray(weight, dtype=np.float32)
    m_core = M_FULL // NCORES
    in_maps = [
        {"x": x[c * m_core : (c + 1) * m_core], "weight": weight}
        for c in range(NCORES)
    ]
    res = run_bass_kernel_spmd(_NC, in_maps, list(range(NCORES)))
    return np.concatenate([res.results[c]["out"] for c in range(NCORES)], axis=0)
